# revision 20
# baseline (speedup 1.0000x reference)
"""Locally-connected 2d (3x3, pad 1) + bias + LeakyReLU(0.1) on 8 trn2 cores.

Strategy
--------
out[n, o, oh, ow] = sum_{c,kh,kw} x[n, c, oh+kh-1, ow+kw-1] * W[o, c, oh, ow, kh*3+kw]

The weight (1, 256, 1024, 7, 7, 9) = 462 MB fp32 dominates all traffic and each
element is used exactly N=32 times, so the kernel sits at the HBM/PE ridge.  We:

  * shard out-channels 8-ways (32 per core) so each core streams 1/8 of W,
  * cast W to fp8 e3m4 scaled by 128 on the host (quarters the dominant
    traffic; ~1.3% rel err), x to bf16 scaled by 1/128 (exact power-of-2
    compensation); the PE accepts mixed bf16 stationary x fp8 moving,
  * skip (location, tap) pairs that read zero padding (361/441 valid -> -18%),
  * keep x stationary in the PE array (lhsT = x[c_chunk, pixel] of shape
    (K=128 c, M=32 n)) and stream host-packed weight columns through the
    moving port: one matmul per (pixel, c_chunk, kh-tap) covering the
    (ow-window x 32 out-ch) output columns it feeds,
  * run contraction chunk c on PE column-tile group c%4 (tile_position
    (0, 32g)): up to 4 concurrent matmul streams in the 128x128 array since
    each uses only 32 array columns (M=32),
  * accumulate in one resident PSUM tile per output row (128 partitions =
    4 groups x 32 n, 512 cols = a full private 2KB bank, so the first
    matmul's start=True bank-clear replaces an explicit zero pass),
  * epilogue per row: ACT copies the 128-partition partials to SBUF bf16, a
    fold matmul (F[k,m] = k%32==m) reduces the 4 group partials, then DVE
    adds the host-broadcast bias and applies LeakyReLU as max(0.1*t, t).

Everything is SPMD-uniform: all per-core differences live in input *content*
(the packed weight / bias), never in shapes or program structure.
"""

import sys

import numpy as np

if "/opt/trn_rl_repo" not in sys.path:
    sys.path.insert(0, "/opt/trn_rl_repo")

import ml_dtypes

# ---------------------------------------------------------------- constants
N = 32
C_IN = 1024
H = W = 7
C_OUT = 256
OH = OW = 7
KH = KW = 3
NCORES = 8
O_SH = C_OUT // NCORES          # 32 out-channels per core
P = 128                          # SBUF partitions
NCHUNK = C_IN // P               # 8 contraction chunks
OH_BLOCK = 512                   # psum cols per oh row: full 2KB bank per tile
                                 # (224 real + pad) so start=True bank-clear
                                 # on the first matmul into a tile is private
REAL_BLOCK = OW * O_SH           # 224
PSUM_COLS = OH * OH_BLOCK        # 1792
OUT_COLS = OH * REAL_BLOCK       # 1568
X_COLS = NCHUNK * H * W * N      # 12544
NEG_SLOPE = 0.1
DMA_GROUP = 2                    # pixels per weight DMA (~0.5 MB each)
W_SCALE = 128.0                  # w*128 fits e3m4 (max |w*s| ~ 10.2 < 15.5);
                                 # compensated exactly by x/128 in bf16
NGROUP = 4                       # PE column-tile groups (32 array cols each)


def _schedule():
    """Per input pixel: valid kh taps and the ascending ow window it feeds."""
    pixels = []
    for ih in range(H):
        for iw in range(W):
            i_list = [i for i in range(KH) if 0 <= ih + 1 - i <= OH - 1]
            ow_list = [ow for ow in range(iw - 1, iw + 2) if 0 <= ow <= OW - 1]
            pixels.append((ih, iw, i_list, ow_list))
    return pixels


_PIXELS = _schedule()
TOTAL_COLS = sum(NCHUNK * len(i) * len(o) * O_SH for _, _, i, o in _PIXELS)  # 92416


# ---------------------------------------------------------------- host packing
def _pack_weight(weight):
    """-> list of 8 arrays (128, TOTAL_COLS) bf16, one per core.

    Column order: pixel-major, then (chunk, kh-tap, ow asc, o).  Row p of
    chunk k holds input channel c = k*128 + p.
    """
    W0 = np.asarray(weight)[0] * W_SCALE                         # (256,1024,7,7,9)
    Wt = np.ascontiguousarray(np.transpose(W0, (1, 0, 2, 3, 4)))  # (c,o,oh,ow,k)
    per_core = [[] for _ in range(NCORES)]
    for ih, iw, i_list, ow_list in _PIXELS:
        ohs, ows, ks = [], [], []
        for i in i_list:
            for ow in ow_list:
                ohs.append(ih + 1 - i)
                ows.append(ow)
                ks.append(i * KW + (iw + 1 - ow))
        B = Wt[:, :, ohs, ows, ks]                    # (1024, 256, npair)
        npair = len(ohs)
        B = B.reshape(NCHUNK, P, C_OUT, npair)
        B = np.transpose(B, (1, 0, 3, 2))             # (p, chunk, pair, o)
        for c in range(NCORES):
            per_core[c].append(
                B[..., c * O_SH:(c + 1) * O_SH].reshape(P, -1))
    return [
        np.ascontiguousarray(np.concatenate(a, axis=1)).astype(ml_dtypes.float8_e3m4)
        for a in per_core
    ]


def _pack_x(x):
    """-> (128, X_COLS) bf16; free index = (chunk*49 + pixel)*32 + n."""
    xt = np.transpose(np.asarray(x), (1, 2, 3, 0))    # (c, h, w, n)
    xt = xt.reshape(NCHUNK, P, H * W, N)
    xt = np.transpose(xt, (1, 0, 2, 3)).reshape(P, X_COLS)
    return np.ascontiguousarray(xt / W_SCALE).astype(ml_dtypes.bfloat16)


def _pack_bias(bias, core):
    b = np.asarray(bias)[0, core * O_SH:(core + 1) * O_SH]   # (32, 7, 7)
    cols = np.transpose(b, (1, 2, 0)).reshape(OUT_COLS)      # (oh, ow, o)
    return np.ascontiguousarray(
        np.broadcast_to(cols[None, :], (N, OUT_COLS))).astype(np.float32)


# ---------------------------------------------------------------- bass program
_PROGRAMS = {}


def _build_program(loop_iters=1):
    """loop_iters>1 wraps the whole body in a device-side For_i so that HW
    exec time can be measured by differencing (axon dispatch is ~100ms)."""
    import contextlib

    import concourse.bacc as bacc
    import concourse.tile as tile
    from concourse import mybir

    nc = bacc.Bacc("TRN2", target_bir_lowering=False, debug=False,
                   num_devices=NCORES)
    w_d = nc.dram_tensor("w", [P, TOTAL_COLS], mybir.dt.float8e3,
                         kind="ExternalInput")
    x_d = nc.dram_tensor("xp", [P, X_COLS], mybir.dt.bfloat16,
                         kind="ExternalInput")
    b_d = nc.dram_tensor("bias", [N, OUT_COLS], mybir.dt.float32,
                         kind="ExternalInput")
    f_d = nc.dram_tensor("fold", [P, N], mybir.dt.bfloat16,
                         kind="ExternalInput")
    o_d = nc.dram_tensor("out", [N, OUT_COLS], mybir.dt.float32,
                         kind="ExternalOutput")

    with tile.TileContext(nc) as tc:
        with (
            tc.tile_pool(name="cpool", bufs=1) as cpool,
            tc.tile_pool(name="wpool", bufs=6) as wpool,
            tc.tile_pool(name="ppool", bufs=1, space="PSUM") as ppool,
            tc.tile_pool(name="opool", bufs=1) as opool,
        ):
            x_sb = cpool.tile([P, X_COLS], mybir.dt.bfloat16)
            nc.sync.dma_start(x_sb[:], x_d[:])
            bias_sb = cpool.tile([N, OUT_COLS], mybir.dt.float32)
            nc.sync.dma_start(bias_sb[:], b_d[:])
            fold_sb = cpool.tile([P, N], mybir.dt.bfloat16)
            nc.sync.dma_start(fold_sb[:], f_d[:])

            if loop_iters > 1:
                loop_cm = tc.For_i(0, loop_iters, 1,
                                   hint_engines=(mybir.EngineType.PE,))
            else:
                loop_cm = contextlib.nullcontext()

            with loop_cm:
                # One PSUM tile per output row; 128 partitions = 4 column-tile
                # groups x 32 batch rows.  Contraction chunk c runs on PE
                # column-group c%4 (tile_position=(0, 32g)), so up to 4
                # matmul streams execute concurrently in the array.  Each
                # tile owns a full 2KB PSUM bank, so the first matmul into it
                # each iteration carries start=True: the bank-wide clear
                # marks every byte pending-zero and later matmuls purely
                # accumulate -- no explicit zero-fill pass needed.
                psums = [ppool.tile([P, OH_BLOCK], mybir.dt.float32,
                                    name=f"psum{oh}", tag=f"psum{oh}")
                         for oh in range(OH)]
                psum2 = ppool.tile([N, OH_BLOCK], mybir.dt.float32,
                                   name="psum_fold", tag="psum_fold")
                started = set()

                foldbuf = opool.tile([P, OUT_COLS], mybir.dt.bfloat16)
                tmp = opool.tile([N, OUT_COLS], mybir.dt.float32)
                out_sb = opool.tile([N, OUT_COLS], mybir.dt.float32)

                def epilogue(oh):
                    # Reduce the 4 group partials (partition blocks of 32)
                    # with a fold matmul: F[k, m] = (k % 32 == m), then
                    # t = fold + bias ; out = max(0.1*t, t)
                    fv = foldbuf[:, oh * REAL_BLOCK:(oh + 1) * REAL_BLOCK]
                    nc.scalar.copy(fv, psums[oh][:, :REAL_BLOCK])
                    nc.tensor.matmul(
                        psum2[:, :REAL_BLOCK], fold_sb[:], fv,
                        start=True, stop=True, skip_group_check=True,
                        tile_position=(0, 0))
                    tv = tmp[:, oh * REAL_BLOCK:(oh + 1) * REAL_BLOCK]
                    bv = bias_sb[:, oh * REAL_BLOCK:(oh + 1) * REAL_BLOCK]
                    ov = out_sb[:, oh * REAL_BLOCK:(oh + 1) * REAL_BLOCK]
                    nc.vector.tensor_add(tv, psum2[:, :REAL_BLOCK], bv)
                    nc.vector.scalar_tensor_tensor(
                        ov, tv, NEG_SLOPE, tv,
                        op0=mybir.AluOpType.mult, op1=mybir.AluOpType.max)
                    # out goes on the ACT HWDGE ring so its HBM-write
                    # completion latency never stalls the weight stream
                    nc.scalar.dma_start(
                        o_d[:, oh * REAL_BLOCK:(oh + 1) * REAL_BLOCK], ov)

                col = 0
                npix = len(_PIXELS)
                groups = [list(range(g, min(g + DMA_GROUP, npix)))
                          for g in range(0, npix, DMA_GROUP)]
                for group in groups:
                    gcols = sum(NCHUNK * len(_PIXELS[p][2]) *
                                len(_PIXELS[p][3]) * O_SH for p in group)
                    wt = wpool.tile([P, gcols], mybir.dt.float8e3, tag="w")
                    nc.sync.dma_start(wt[:], w_d[:, col:col + gcols])
                    wc = 0
                    for pix in group:
                        ih, iw, i_list, ow_list = _PIXELS[pix]
                        ncols = len(ow_list) * O_SH
                        ow0 = ow_list[0]
                        for chunk in range(NCHUNK):
                            s = (chunk * H * W + pix) * N
                            lhs = x_sb[:, s:s + N]
                            g = chunk % NGROUP
                            for i in i_list:
                                oh = ih + 1 - i
                                # the bank-clear covers only the written
                                # partitions, so each group needs its own
                                # first-touch start=True
                                st = (oh, g) not in started
                                started.add((oh, g))
                                nc.tensor.matmul(
                                    psums[oh][32 * g:32 * g + N,
                                              ow0 * O_SH:ow0 * O_SH + ncols],
                                    lhs, wt[:, wc:wc + ncols],
                                    start=st, stop=False,
                                    skip_group_check=True,
                                    tile_position=(0, 32 * g))
                                wc += ncols
                        if iw == W - 1:
                            # row ih done: output row ih-1 is complete
                            if ih >= 1:
                                epilogue(ih - 1)
                            if ih == H - 1:
                                epilogue(ih)
                    assert wc == gcols
                    col += gcols
                assert col == TOTAL_COLS

    nc.finalize()
    return nc


def _get_program(loop_iters=1):
    if loop_iters not in _PROGRAMS:
        _PROGRAMS[loop_iters] = _build_program(loop_iters)
    return _PROGRAMS[loop_iters]


# ---------------------------------------------------------------- pjrt runner
class _Runner:
    """Compiled SPMD executor with a persistent jit cache.

    Mirrors concourse.bass2jax.run_bass_via_pjrt's multi-core path, but keeps
    the jitted callable (and optionally device-resident inputs) across calls
    so the kernel can be re-executed without re-tracing / re-transferring.
    """

    def __init__(self, nc):
        import jax
        from jax.sharding import Mesh, PartitionSpec
        from jax.experimental.shard_map import shard_map
        from concourse import bass2jax, mybir

        bass2jax.install_neuronx_cc_hook()
        self.jax = jax
        partition_name = (nc.partition_id_tensor.name
                          if nc.partition_id_tensor else None)
        in_names, out_names, out_avals = [], [], []
        zero_outs = []
        for alloc in nc.m.functions[0].allocations:
            if not isinstance(alloc, mybir.MemoryLocationSet):
                continue
            name = alloc.memorylocations[0].name
            if alloc.kind == "ExternalInput":
                if name != partition_name:
                    in_names.append(name)
            elif alloc.kind == "ExternalOutput":
                out_names.append(name)
                shape = tuple(alloc.tensor_shape)
                dtype = mybir.dt.np(alloc.dtype)
                out_avals.append(jax.core.ShapedArray(shape, dtype))
                zero_outs.append(np.zeros(shape, dtype))
        self.in_names = list(in_names)
        self.out_names = out_names
        self.out_avals = out_avals
        self.zero_outs = zero_outs
        n_params = len(in_names)
        n_outs = len(out_avals)
        all_in_names = list(in_names) + list(out_names)
        if partition_name is not None:
            all_in_names.append(partition_name)

        def _body(*args):
            operands = list(args)
            if partition_name is not None:
                operands.append(bass2jax.partition_id_tensor())
            outs = bass2jax._bass_exec_p.bind(
                *operands,
                out_avals=tuple(out_avals),
                in_names=tuple(all_in_names),
                out_names=tuple(out_names),
                lowering_input_output_aliases=(),
                sim_require_finite=True,
                sim_require_nnan=True,
                nc=nc,
            )
            return tuple(outs)

        devices = jax.devices()[:NCORES]
        self.mesh = Mesh(np.asarray(devices), ("core",))
        self.pspec = PartitionSpec("core")
        in_specs = (self.pspec,) * (n_params + n_outs)
        out_specs = (self.pspec,) * n_outs
        # No donation: the kernel writes every element of its outputs, so the
        # (required-by-signature) zero buffers are never actually read and can
        # stay device-resident across calls.
        self.fn = jax.jit(
            shard_map(_body, mesh=self.mesh, in_specs=in_specs,
                      out_specs=out_specs, check_rep=False),
            keep_unused=True)

    def stage_inputs(self, in_maps):
        """Concatenate per-core inputs and push them to the devices once."""
        from jax.sharding import NamedSharding
        concat = [
            np.concatenate([np.asarray(in_maps[c][n]) for c in range(NCORES)],
                           axis=0)
            for n in self.in_names
        ]
        concat += [np.zeros((NCORES * z.shape[0], *z.shape[1:]), z.dtype)
                   for z in self.zero_outs]
        sh = NamedSharding(self.mesh, self.pspec)
        return [self.jax.device_put(a, sh) for a in concat]

    def execute(self, staged):
        outs = self.fn(*staged)
        return outs

    def results(self, outs):
        out_np = [np.asarray(o) for o in outs]
        return [
            {n: out_np[i].reshape(NCORES, *self.out_avals[i].shape)[c]
             for i, n in enumerate(self.out_names)}
            for c in range(NCORES)
        ]


_RUNNERS = {}


def _get_runner(loop_iters=1):
    if loop_iters not in _RUNNERS:
        _RUNNERS[loop_iters] = _Runner(_get_program(loop_iters))
    return _RUNNERS[loop_iters]


# ---------------------------------------------------------------- entry points
def _fold_matrix():
    f = np.zeros((P, N), dtype=np.float32)
    f[np.arange(P), np.arange(P) % N] = 1.0
    return f.astype(ml_dtypes.bfloat16)


def _in_maps(inputs):
    w_cores = _pack_weight(inputs["weight"])
    xp = _pack_x(inputs["x"])
    fold = _fold_matrix()
    return [
        {"w": w_cores[c], "xp": xp, "bias": _pack_bias(inputs["bias"], c),
         "fold": fold}
        for c in range(NCORES)
    ]


def _assemble(results):
    parts = []
    for c in range(NCORES):
        o = results[c]["out"].reshape(N, OH, OW, O_SH)
        parts.append(np.transpose(o, (0, 3, 1, 2)))
    return np.concatenate(parts, axis=1).astype(np.float32)


def _run(inputs, trace=False, trace_cores=None):
    r = _get_runner()
    staged = r.stage_inputs(_in_maps(inputs))
    outs = r.execute(staged)
    return _assemble(r.results(outs)), None


def kernel(x, weight, bias):
    out, _ = _run({"x": x, "weight": weight, "bias": bias})
    return out

